# revision 28
# baseline (speedup 1.0000x reference)
"""DecoupledContrastiveLoss on 8 Trainium2 NeuronCores.

Strategy (data parallel over batch rows, per sharding hint):
  - Host: stable-sort rows by match_id (makes the positive mask a narrow
    band around the diagonal), L2-normalize rows, transpose both feature
    matrices to [D, B] so the contraction dim lands on SBUF partitions,
    and ship each core a column-rotated copy (rotation by core*1024 puts
    the core's own diagonal block at local columns [0, 1024), so one SPMD
    program serves all cores).
  - Device (per core, fp32r matmuls): 4 row-sharded [1024, 8192] similarity
    passes (v2t, t2v, v@v.T, t@t.T). Each sim chunk goes PSUM -> ACT
    exp(x/T) with fused row-sum accumulation. DVE computes top-8
    max+indices per half-row (v2t/t2v) and the masked positive sums over
    the 256-wide diagonal band (is_equal vs ids + multiply-reduce).
    Instance passes extract exp(diag) via an identity multiply-reduce.
  - Host: combines per-core/per-half partials, computes the log-space
    losses, refines argmax among the 16 device candidates with exact
    dots, and assembles the 9 reference outputs.
"""
import sys

if "/opt/trn_rl_repo" not in sys.path:
    sys.path.insert(0, "/opt/trn_rl_repo")

import numpy as np

import concourse.bacc as bacc
import concourse.tile as tile
import concourse.mybir as mybir
from concourse.bass_utils import run_bass_kernel_spmd

DT = mybir.dt

N_CORES = 8
B = 8192
D = 512
BL = B // N_CORES          # 1024 rows per core
NT = BL // 128             # 8 i-tiles per core
HALF = B // 2              # 4096 columns per phase
TEMP = 0.07
T_INV = 1.0 / TEMP
WIN = 256                  # positive-band window width (max group size 8 << 64)

_program = None
_last_in_maps = None


def _build_program(repeat=1, mov_bufs=8, e_bufs=2, es_bufs=2, ps_bufs=4):
    nc = bacc.Bacc("TRN2", target_bir_lowering=False, debug=False,
                   num_devices=N_CORES)

    vmov = nc.dram_tensor("vmov", [D, B], DT.float32r, kind="ExternalInput").ap()
    tmov = nc.dram_tensor("tmov", [D, B], DT.float32r, kind="ExternalInput").ap()
    ids_win = nc.dram_tensor("ids_win", [NT, WIN], DT.float32, kind="ExternalInput").ap()
    ids_loc = nc.dram_tensor("ids_loc", [128, NT], DT.float32, kind="ExternalInput").ap()
    ident = nc.dram_tensor("ident", [128, 128], DT.float32, kind="ExternalInput").ap()

    def out_t(name, w, dtype=DT.float32):
        return nc.dram_tensor(name, [BL, w], dtype, kind="ExternalOutput").ap()

    outs = {}
    for nm in ("v2t", "t2v"):
        outs[nm + "_tot"] = out_t(nm + "_tot", 2)
        outs[nm + "_pos"] = out_t(nm + "_pos", 2)
        outs[nm + "_max"] = out_t(nm + "_max", 16)
        outs[nm + "_idx"] = out_t(nm + "_idx", 16, DT.uint32)
    for nm in ("vv", "tt"):
        outs[nm + "_tot"] = out_t(nm + "_tot", 2)
        outs[nm + "_diag"] = out_t(nm + "_diag", 1)

    with tile.TileContext(nc) as tc:
        with tc.tile_pool(name="consts", bufs=1) as cpool, \
             tc.tile_pool(name="mov", bufs=mov_bufs) as mpool, \
             tc.tile_pool(name="eblk", bufs=e_bufs) as epool, \
             tc.tile_pool(name="esc", bufs=es_bufs) as escpool, \
             tc.tile_pool(name="small", bufs=3) as spool, \
             tc.tile_pool(name="gmp", bufs=2) as gmpool, \
             tc.tile_pool(name="psum", bufs=ps_bufs, space="PSUM") as pspool:

            def load_mov(mat, half):
                mov_dram = tmov if mat == "t" else vmov
                mk = [mpool.tile([128, HALF], DT.float32r, name="movk")
                      for _ in range(4)]
                # q-outer: the first 512-col chunk needs all four k slices,
                # so land the q=0 pieces of every k first
                for q in range(4):
                    for k in range(4):
                        nc.sync.dma_start(
                            mk[k][:, q * 1024:(q + 1) * 1024],
                            mov_dram[k * 128:(k + 1) * 128,
                                     half * HALF + q * 1024:
                                     half * HALF + (q + 1) * 1024])
                return mk

            # phases: (moving matrix, half)
            phases = [("t", 0), ("t", 1), ("v", 0), ("v", 1)] * repeat

            # critical path first: cross stationary (vloc) + phase-0 moving
            vloc = cpool.tile([128, 4 * BL], DT.float32r)
            tloc = cpool.tile([128, 4 * BL], DT.float32r)
            for k in range(4):
                nc.sync.dma_start(vloc[:, k * BL:(k + 1) * BL],
                                  vmov[k * 128:(k + 1) * 128, 0:BL])
            mk0 = load_mov(*phases[0])
            for k in range(4):
                nc.sync.dma_start(tloc[:, k * BL:(k + 1) * BL],
                                  tmov[k * 128:(k + 1) * 128, 0:BL])

            win = cpool.tile([128, NT * WIN], DT.float32)
            for it in range(NT):
                nc.gpsimd.dma_start(win[:, it * WIN:(it + 1) * WIN],
                                    ids_win[it:it + 1, :].partition_broadcast(128))
            idl = cpool.tile([128, NT], DT.float32)
            nc.gpsimd.dma_start(idl[:], ids_loc[:])
            idn = cpool.tile([128, 128], DT.float32)
            nc.gpsimd.dma_start(idn[:], ident[:])
            for pi, (mat, half) in enumerate(phases):
                cross = "v2t" if mat == "t" else "t2v"
                inst = "tt" if mat == "t" else "vv"
                cstat = vloc if mat == "t" else tloc
                istat = tloc if mat == "t" else vloc

                mk = mk0 if pi == 0 else load_mov(mat, half)

                def mm_group(pp, stat, it, g):
                    # fill [128, 1024] psum group g of i-tile it
                    for cc in range(2):
                        for k in range(4):
                            nc.tensor.matmul(
                                pp[:, cc * 512:(cc + 1) * 512],
                                stat[:, k * BL + it * 128: k * BL + it * 128 + 128],
                                mk[k][:, g * 1024 + cc * 512: g * 1024 + (cc + 1) * 512],
                                start=(k == 0), stop=(k == 3))

                for it in range(NT):
                    # ---------- cross-modal i-tile (needs max/idx + pos) ----
                    e = epool.tile([128, HALF], DT.bfloat16, name="e")
                    tp = spool.tile([128, 4], DT.float32, name="tp")
                    for g in range(4):
                        pp = pspool.tile([128, 1024], DT.float32, name="pp")
                        mm_group(pp, cstat, it, g)
                        nc.scalar.activation(
                            e[:, g * 1024:(g + 1) * 1024], pp[:],
                            mybir.ActivationFunctionType.Exp,
                            bias=0.0, scale=T_INV, accum_out=tp[:, g:g + 1])
                    tot1 = spool.tile([128, 1], DT.float32, name="tot1")
                    nc.vector.tensor_reduce(tot1[:], tp[:],
                                            axis=mybir.AxisListType.X,
                                            op=mybir.AluOpType.add)
                    nc.gpsimd.dma_start(
                        outs[cross + "_tot"][it * 128:(it + 1) * 128, half:half + 1],
                        tot1[:])
                    # two-level argmax: 8-wide group maxes, then top-8 groups
                    gm = gmpool.tile([128, HALF // 8], DT.float32, name="gm")
                    nc.vector.tensor_reduce(
                        gm[:], e[:].rearrange("p (g k) -> p g k", k=8),
                        axis=mybir.AxisListType.X, op=mybir.AluOpType.max)
                    mx = spool.tile([128, 8], DT.float32, name="mx")
                    ix = spool.tile([128, 8], DT.uint32, name="ix")
                    nc.vector.max_with_indices(mx[:], ix[:], gm[:])
                    nc.gpsimd.dma_start(
                        outs[cross + "_max"][it * 128:(it + 1) * 128,
                                             half * 8:(half + 1) * 8], mx[:])
                    nc.gpsimd.dma_start(
                        outs[cross + "_idx"][it * 128:(it + 1) * 128,
                                             half * 8:(half + 1) * 8], ix[:])

                    # positive band: local cols [it*128-64, it*128+192) mod B
                    def mask_pos(e_lo, e_hi, w_lo, pos_col):
                        width = e_hi - e_lo
                        msk = spool.tile([128, WIN], DT.float32, name="msk")
                        junk = spool.tile([128, WIN], DT.float32, name="junk")
                        pos1 = spool.tile([128, 1], DT.float32, name="pos1")
                        nc.vector.tensor_scalar(
                            msk[:, 0:width],
                            win[:, it * WIN + w_lo: it * WIN + w_lo + width],
                            idl[:, it:it + 1], None,
                            op0=mybir.AluOpType.is_equal)
                        nc.vector.tensor_tensor(
                            junk[:, 0:width], e[:, e_lo:e_hi], msk[:, 0:width],
                            op=mybir.AluOpType.mult)
                        nc.vector.tensor_reduce(
                            pos1[:], junk[:, 0:width],
                            axis=mybir.AxisListType.X, op=mybir.AluOpType.add)
                        nc.gpsimd.dma_start(
                            outs[cross + "_pos"][it * 128:(it + 1) * 128,
                                                 pos_col:pos_col + 1], pos1[:])

                    if half == 0:
                        if it == 0:
                            mask_pos(0, 192, 64, 0)       # cols [0, 192)
                        else:
                            mask_pos(it * 128 - 64, it * 128 + 192, 0, 0)
                    elif it == 0:
                        mask_pos(HALF - 64, HALF, 0, 1)    # wrap: cols [B-64, B)

                    # ---------- instance i-tile (tot + diag only) ----------
                    tpi = spool.tile([128, 4], DT.float32, name="tpi")
                    for g in range(4):
                        pp = pspool.tile([128, 1024], DT.float32, name="pp")
                        mm_group(pp, istat, it, g)
                        es = escpool.tile([128, 1024], DT.float32, name="es")
                        nc.scalar.activation(
                            es[:], pp[:], mybir.ActivationFunctionType.Exp,
                            bias=0.0, scale=T_INV, accum_out=tpi[:, g:g + 1])
                        if half == 0 and g == 0:
                            junkd = spool.tile([128, 128], DT.float32, name="junkd")
                            diag1 = spool.tile([128, 1], DT.float32, name="diag1")
                            nc.vector.tensor_tensor(
                                junkd[:], es[:, it * 128:it * 128 + 128], idn[:],
                                op=mybir.AluOpType.mult)
                            nc.vector.tensor_reduce(
                                diag1[:], junkd[:],
                                axis=mybir.AxisListType.X, op=mybir.AluOpType.add)
                            nc.gpsimd.dma_start(
                                outs[inst + "_diag"][it * 128:(it + 1) * 128, 0:1],
                                diag1[:])
                    toti = spool.tile([128, 1], DT.float32, name="toti")
                    nc.vector.tensor_reduce(toti[:], tpi[:],
                                            axis=mybir.AxisListType.X,
                                            op=mybir.AluOpType.add)
                    nc.gpsimd.dma_start(
                        outs[inst + "_tot"][it * 128:(it + 1) * 128, half:half + 1],
                        toti[:])
    nc.compile()
    return nc


def _get_program():
    global _program
    if _program is None:
        _program = _build_program()
    return _program


def kernel(vision_features, text_features, match_ids):
    v = np.asarray(vision_features, dtype=np.float32)
    t = np.asarray(text_features, dtype=np.float32)
    ids = np.asarray(match_ids)

    # ---- host prep: sort by id, normalize, transpose ----
    perm = np.argsort(ids, kind="stable")
    ids_s = ids[perm].astype(np.int64)
    v_s = v[perm]
    t_s = t[perm]
    vn = (v_s / np.linalg.norm(v_s, axis=1, keepdims=True)).astype(np.float32)
    tn = (t_s / np.linalg.norm(t_s, axis=1, keepdims=True)).astype(np.float32)
    vT = np.ascontiguousarray(vn.T)   # [D, B]
    tT = np.ascontiguousarray(tn.T)

    # group ranges in sorted order
    ids_f = ids_s.astype(np.float32)
    change = np.nonzero(np.diff(ids_s))[0] + 1
    starts = np.concatenate([[0], change])
    ends = np.concatenate([change, [B]])
    cnt = ends - starts
    num_pos = int((cnt.astype(np.int64) ** 2).sum())
    assert cnt.max() <= 64, "positive band wider than window"

    in_maps = []
    for d in range(N_CORES):
        sl = slice(d * BL, (d + 1) * BL)
        vrot = np.roll(vT, -d * BL, axis=1)
        trot = np.roll(tT, -d * BL, axis=1)
        ids_win = np.empty((NT, WIN), np.float32)
        for it in range(NT):
            cols = (np.arange(it * 128 - 64, it * 128 + 192) + d * BL) % B
            ids_win[it] = ids_f[cols]
        ids_loc = ids_f[sl].reshape(NT, 128).T.copy()  # [128, NT]
        in_maps.append({
            "vmov": vrot, "tmov": trot,
            "ids_win": ids_win, "ids_loc": ids_loc,
            "ident": np.eye(128, dtype=np.float32),
        })

    global _last_in_maps
    _last_in_maps = in_maps
    nc = _get_program()
    res = run_bass_kernel_spmd(nc, in_maps, list(range(N_CORES)))

    def gather(name):
        return np.concatenate([res.results[c][name] for c in range(N_CORES)], axis=0)

    out = {k: gather(k) for k in
           ["v2t_tot", "v2t_pos", "v2t_max", "v2t_idx",
            "t2v_tot", "t2v_pos", "t2v_max", "t2v_idx",
            "vv_tot", "vv_diag", "tt_tot", "tt_diag"]}

    # ---- losses (all rows valid: every row has >=1 pos and >=1 neg) ----
    f64 = np.float64
    v2t_tot = out["v2t_tot"].astype(f64).sum(1)
    t2v_tot = out["t2v_tot"].astype(f64).sum(1)

    def pos_sum(a):
        # column 1 is only written for each core's first i-tile (the band
        # wrap); everything else is undefined memory — mask it out.
        s = a.astype(f64)[:, 0].copy()
        for c in range(N_CORES):
            lo = c * BL
            s[lo:lo + 128] += a[lo:lo + 128, 1].astype(f64)
        return s

    v2t_pos = pos_sum(out["v2t_pos"])
    t2v_pos = pos_sum(out["t2v_pos"])
    v2t_loss = (np.log(v2t_tot) - np.log(v2t_pos)).sum() / num_pos
    t2v_loss = (np.log(t2v_tot) - np.log(t2v_pos)).sum() / num_pos
    cross = 0.5 * (v2t_loss + t2v_loss)

    vv_tot = out["vv_tot"].astype(f64).sum(1)
    tt_tot = out["tt_tot"].astype(f64).sum(1)
    v_inst = (np.log(vv_tot) - np.log(out["vv_diag"].astype(f64)[:, 0])).mean()
    t_inst = (np.log(tt_tot) - np.log(out["tt_diag"].astype(f64)[:, 0])).mean()

    total = cross + 0.5 * v_inst + 0.5 * t_inst

    # ---- accuracy: refine argmax among the 16 device candidates ----
    core_of_row = np.repeat(np.arange(N_CORES), BL)

    def refine(idx, a_s, b_s):
        # idx: [B, 16] top-8 group indices per half (groups of 8 columns);
        # expand to the 128 member columns and take the exact-fp32 argmax.
        gidx = idx.astype(np.int64)
        gidx[:, 8:] += HALF // 8
        loc = (gidx[:, :, None] * 8 + np.arange(8)).reshape(B, 128)
        g = (loc + core_of_row[:, None] * BL) % B      # global sorted col
        best = np.empty(B, np.int64)
        for lo in range(0, B, 512):
            hi = lo + 512
            cand = b_s[g[lo:hi]]                       # [512, 128, D]
            sims = np.matmul(cand, a_s[lo:hi, :, None])[:, :, 0]
            best[lo:hi] = g[np.arange(lo, hi), sims.argmax(1)]
        return best

    v2t_pred_s = refine(out["v2t_idx"], vn, tn)
    t2v_pred_s = refine(out["t2v_idx"], tn, vn)

    # map sorted-space preds back to original indexing
    ids_orig = ids.astype(np.int64)
    order = np.argsort(ids_orig, kind="stable")
    first_occ_sorted_pos = np.searchsorted(ids_orig[order], ids_orig)
    targets = order[first_occ_sorted_pos]              # first orig idx with same id

    pred_v2t = np.empty(B, np.int64)
    pred_v2t[perm] = perm[v2t_pred_s]
    pred_t2v = np.empty(B, np.int64)
    pred_t2v[perm] = perm[t2v_pred_s]
    v2t_acc = (pred_v2t == targets).mean()
    t2v_acc = (pred_t2v == targets).mean()

    r = np.float32
    return (r(total), r(cross), r(v2t_loss), r(t2v_loss),
            r(v_inst), r(t_inst), r(v2t_acc), r(t2v_acc),
            r((v2t_acc + t2v_acc) / 2.0))
